# revision 1
# baseline (speedup 1.0000x reference)
"""Bipartite GNN message passing on 8 Trainium2 NeuronCores.

Math reformulation: relu(h[idx] @ W + b) == relu(h @ W + b)[idx], so each
direction-layer is: per-node message MLP (z) -> gather z rows by edge ->
segment-sum -> update MLP.  Sharding: aggregation-side nodes are split into
8 contiguous ranges (one per core); each core owns ALL edges targeting its
range, so it computes complete aggregates locally (no AllReduce).  Only the
small per-shard z tensors are AllGathered (fp16) each direction-layer.

Segment-sum on the tensor engine: edges sorted by (gather-table, subwindow);
for each 128-edge chunk a one-hot S[e,j] = (dst_local[e] == j) matrix (built
with one DVE is_equal against a constant iota tile) turns the segment sum
into  psum[64f, 64dst] += gathered_z[128e, 64f].T @ S[128e, 64dst],
accumulated per 512-dst PSUM window and flushed additively to SBUF.

Gathers use dma_gather (one SWDGE instruction per ~8k rows, int16 indices
into <=25088-row table slices of the AllGathered z, 256B padded fp16 rows).

SPMD: one NEFF for all 8 cores, so the chunk schedule (chunks per
(table, subwindow)) is the max over cores; cores pad with (idx=0,
dst_local=sentinel) edges that contribute zero.
"""
import numpy as np

D = 64
CHUNK = 128
SUB = 64
WIN = 512
SENT = 999.0
N_CORES = 8
GMAX = 8192


class Cfg:
    def __init__(self, ns_pad, nd_pad, n_tab_s, n_tab_d):
        self.NS_PAD, self.ND_PAD = ns_pad, nd_pad
        self.SRC_SH, self.DST_SH = ns_pad // N_CORES, nd_pad // N_CORES
        self.N_TAB_S, self.N_TAB_D = n_tab_s, n_tab_d
        self.TAB_S = ns_pad // n_tab_s
        self.TAB_D = nd_pad // n_tab_d
        assert self.TAB_S <= 32767 and self.TAB_D <= 32767
        assert self.SRC_SH % CHUNK == 0 and self.DST_SH % CHUNK == 0


REAL_CFG = Cfg(100352, 50176, 4, 2)


def _build_plan(gather_idx, seg_idx, table_rows, n_tables, shard):
    """SPMD-uniform edge plan for one direction.

    Returns dict with per-core idx16 [128, TOT/16] int16, dw [128, TOT/128]
    fp16, plus uniform visits [(q, w, [k_per_sub])] and gathers
    [(q, slot0, nslots)].
    """
    n_sub_tot = shard // SUB
    core_of = seg_idx // shard
    per_core = []
    Kmax = np.zeros((n_tables, n_sub_tot), np.int64)
    for c in range(N_CORES):
        m = core_of == c
        g = gather_idx[m]
        s = seg_idx[m] - c * shard
        q = (g // table_rows).astype(np.int64)
        subg = s // SUB
        key = q * n_sub_tot + subg
        order = np.argsort(key, kind="stable")
        g, s, q, subg = g[order], s[order], q[order], subg[order]
        per_core.append((g % table_rows, s, q, subg))
        cnt = np.zeros((n_tables, n_sub_tot), np.int64)
        np.add.at(cnt, (q, subg), 1)
        Kmax = np.maximum(Kmax, (cnt + CHUNK - 1) // CHUNK)

    slots_per_group = Kmax * CHUNK
    flat = slots_per_group.reshape(-1)
    starts = np.concatenate([[0], np.cumsum(flat)[:-1]]).reshape(n_tables, n_sub_tot)
    total = int(flat.sum())
    assert total % CHUNK == 0

    idx16_list, dw_list = [], []
    for c in range(N_CORES):
        g, s, q, subg = per_core[c]
        G = np.zeros(total, np.int32)
        DW = np.full(total, SENT, np.float32)
        kk = q * n_sub_tot + subg
        bounds = np.flatnonzero(np.diff(kk)) + 1
        for grp in np.split(np.arange(len(g)), bounds):
            if len(grp) == 0:
                continue
            qq, sg = int(q[grp[0]]), int(subg[grp[0]])
            st = int(starts[qq, sg])
            n = len(grp)
            G[st:st + n] = g[grp]
            DW[st:st + n] = s[grp] - sg * SUB
        # idx16: j -> [16k + j%16, j//16] replicated for 8 gpsimd cores
        i16 = np.empty((128, total // 16), np.int16)
        base = G.astype(np.int16).reshape(total // 16, 16).T  # [16, T/16]
        for k in range(8):
            i16[16 * k:16 * (k + 1)] = base
        dw = DW.astype(np.float16).reshape(total // CHUNK, CHUNK).T  # [128, T/128]
        idx16_list.append(i16)
        dw_list.append(np.ascontiguousarray(dw))

    n_win = (shard + WIN - 1) // WIN
    nsub_per_win = WIN // SUB
    visits = []
    for q in range(n_tables):
        for w in range(n_win):
            lo = w * nsub_per_win
            hi = min(lo + nsub_per_win, n_sub_tot)
            ks = [int(Kmax[q, sg]) for sg in range(lo, hi)]
            visits.append((q, w, ks))

    gathers = []
    for q in range(n_tables):
        q_lo = int(starts[q, 0])
        q_hi = int(starts[q, n_sub_tot - 1] + slots_per_group[q, n_sub_tot - 1])
        p = q_lo
        while p < q_hi:
            n = min(GMAX, q_hi - p)
            gathers.append((q, p, n))
            p += n
    return dict(idx16=idx16_list, dw=dw_list, visits=visits, gathers=gathers,
                total=total)


def _host_prep(cfg, inputs):
    f32 = np.float32
    x_src = np.asarray(inputs["x_src"], f32)
    x_dst = np.asarray(inputs["x_dst"], f32)
    src_idx = np.asarray(inputs["src_idx"]).astype(np.int64)
    dst_idx = np.asarray(inputs["dst_idx"]).astype(np.int64)
    L = np.asarray(inputs["W_msg_sd"]).shape[0]

    ns, nd = x_src.shape[0], x_dst.shape[0]
    xs = np.zeros((cfg.NS_PAD, D), f32)
    xs[:ns] = x_src
    xd = np.zeros((cfg.ND_PAD, D), f32)
    xd[:nd] = x_dst

    plan_sd = _build_plan(src_idx, dst_idx, cfg.TAB_S, cfg.N_TAB_S, cfg.DST_SH)
    plan_ds = _build_plan(dst_idx, src_idx, cfg.TAB_D, cfg.N_TAB_D, cfg.SRC_SH)

    def stack_wb(wk, bk):
        w = np.asarray(inputs[wk], f32)
        b = np.asarray(inputs[bk], f32)
        out = np.empty((L, D + 1, D), np.float16)
        out[:, :D] = w.astype(np.float16)
        out[:, D] = b.astype(np.float16)
        return out

    host = dict(
        L=L,
        Wbm_sd=stack_wb("W_msg_sd", "b_msg_sd"),
        Wbm_ds=stack_wb("W_msg_ds", "b_msg_ds"),
        Wu_dst=np.asarray(inputs["W_upd_dst"], f32).astype(np.float16),
        Wu_src=np.asarray(inputs["W_upd_src"], f32).astype(np.float16),
        bu_dst=np.asarray(inputs["b_upd_dst"], f32)[:, :, None],
        bu_src=np.asarray(inputs["b_upd_src"], f32)[:, :, None],
        Win_src=np.asarray(inputs["W_in_src"], f32),
        Win_dst=np.asarray(inputs["W_in_dst"], f32),
        bin_src=np.asarray(inputs["b_in_src"], f32)[:, None],
        bin_dst=np.asarray(inputs["b_in_dst"], f32)[:, None],
        iota=np.tile(np.arange(SUB, dtype=np.float16), (128, 1)),
        xsT=[np.ascontiguousarray(xs[c * cfg.SRC_SH:(c + 1) * cfg.SRC_SH].T)
             for c in range(N_CORES)],
        xdT=[np.ascontiguousarray(xd[c * cfg.DST_SH:(c + 1) * cfg.DST_SH].T)
             for c in range(N_CORES)],
        plan_sd=plan_sd, plan_ds=plan_ds,
    )
    return host


def _build_nc(cfg, host, reps=1):
    import concourse.bass as bass
    import concourse.tile as tile
    from concourse import bacc, mybir

    dt = mybir.dt
    L = host["L"]
    plan_sd, plan_ds = host["plan_sd"], host["plan_ds"]
    TOT_SD, TOT_DS = plan_sd["total"], plan_ds["total"]

    nc = bacc.Bacc("TRN2", target_bir_lowering=False, debug=False,
                   num_devices=N_CORES, num_swdge_queues=4)

    def inp(name, shape, dtype):
        return nc.dram_tensor(name, shape, dtype, kind="ExternalInput").ap()

    xT_src = inp("xT_src", [D, cfg.SRC_SH], dt.float32)
    xT_dst = inp("xT_dst", [D, cfg.DST_SH], dt.float32)
    Win_src = inp("Win_src", [D, D], dt.float32)
    Win_dst = inp("Win_dst", [D, D], dt.float32)
    bin_src = inp("bin_src", [D, 1], dt.float32)
    bin_dst = inp("bin_dst", [D, 1], dt.float32)
    Wbm_sd = inp("Wbm_sd", [L, D + 1, D], dt.float16)
    Wbm_ds = inp("Wbm_ds", [L, D + 1, D], dt.float16)
    Wu_dst = inp("Wu_dst", [L, 2 * D, D], dt.float16)
    Wu_src = inp("Wu_src", [L, 2 * D, D], dt.float16)
    bu_dst = inp("bu_dst", [L, D, 1], dt.float32)
    bu_src = inp("bu_src", [L, D, 1], dt.float32)
    iota_in = inp("iota", [128, SUB], dt.float16)
    idx_sd = inp("idx_sd", [128, TOT_SD // 16], dt.int16)
    dw_sd = inp("dw_sd", [128, TOT_SD // CHUNK], dt.float16)
    idx_ds = inp("idx_ds", [128, TOT_DS // 16], dt.int16)
    dw_ds = inp("dw_ds", [128, TOT_DS // CHUNK], dt.float16)
    out_hd = nc.dram_tensor("out_hd", [D, cfg.DST_SH], dt.float16,
                            kind="ExternalOutput").ap()

    zs_shard = nc.dram_tensor("zs_shard", [cfg.SRC_SH, 128], dt.float16).ap()
    zd_shard = nc.dram_tensor("zd_shard", [cfg.DST_SH, 128], dt.float16).ap()
    zs_full = nc.dram_tensor("zs_full", [cfg.NS_PAD, 128], dt.float16,
                             addr_space="Shared").ap()
    zd_full = nc.dram_tensor("zd_full", [cfg.ND_PAD, 128], dt.float16,
                             addr_space="Shared").ap()

    RELU = mybir.ActivationFunctionType.Relu
    EQ = mybir.AluOpType.is_equal
    rg = [list(range(N_CORES))]

    with tile.TileContext(nc) as tc:
        from contextlib import ExitStack
        with ExitStack() as ctx:
            pers = ctx.enter_context(tc.tile_pool(name="pers", bufs=1))
            ps_agg = ctx.enter_context(
                tc.tile_pool(name="psagg", bufs=4, space="PSUM"))
            ps_mlp = ctx.enter_context(
                tc.tile_pool(name="psmlp", bufs=2, space="PSUM"))
            gath = ctx.enter_context(tc.tile_pool(name="gath", bufs=2))
            idxg = ctx.enter_context(tc.tile_pool(name="idxg", bufs=4))
            spool = ctx.enter_context(tc.tile_pool(name="spool", bufs=4))
            work = ctx.enter_context(tc.tile_pool(name="work", bufs=4))

            h_s = pers.tile([D + 1, cfg.SRC_SH], dt.float16, name="h_s")
            h_d = pers.tile([D + 1, cfg.DST_SH], dt.float16, name="h_d")
            h_t = {"s": h_s, "d": h_d}
            agg_s = pers.tile([D, cfg.SRC_SH], dt.float16)
            agg_d = pers.tile([D, cfg.DST_SH], dt.float16)
            iota_t = pers.tile([128, SUB], dt.float16)
            dw_sd_t = pers.tile([128, TOT_SD // CHUNK], dt.float16)
            dw_ds_t = pers.tile([128, TOT_DS // CHUNK], dt.float16)

            nc.sync.dma_start(out=iota_t[:], in_=iota_in[:])
            nc.sync.dma_start(out=dw_sd_t[:], in_=dw_sd[:])
            nc.sync.dma_start(out=dw_ds_t[:], in_=dw_ds[:])

            w_enc_s = pers.tile([D, D], dt.float32)
            w_enc_d = pers.tile([D, D], dt.float32)
            b_enc_s = pers.tile([D, 1], dt.float32)
            b_enc_d = pers.tile([D, 1], dt.float32)
            nc.sync.dma_start(out=w_enc_s[:], in_=Win_src[:])
            nc.sync.dma_start(out=w_enc_d[:], in_=Win_dst[:])
            nc.sync.dma_start(out=b_enc_s[:], in_=bin_src[:])
            nc.sync.dma_start(out=b_enc_d[:], in_=bin_dst[:])

            wbm_t, wu_t, bu_t = {}, {}, {}
            for l in range(L):
                for key, src in (("sd", Wbm_sd), ("ds", Wbm_ds)):
                    t = pers.tile([D + 1, D], dt.float16, name=f"wbm_{key}{l}")
                    nc.sync.dma_start(out=t[:], in_=src[l])
                    wbm_t[key, l] = t
                for key, src in (("dst", Wu_dst), ("src", Wu_src)):
                    t = pers.tile([2 * D, D], dt.float16, name=f"wu_{key}{l}")
                    nc.sync.dma_start(out=t[:], in_=src[l])
                    wu_t[key, l] = t
                for key, src in (("dst", bu_dst), ("src", bu_src)):
                    t = pers.tile([D, 1], dt.float32, name=f"bu_{key}{l}")
                    nc.sync.dma_start(out=t[:], in_=src[l])
                    bu_t[key, l] = t

            for t in h_t.values():
                nc.vector.memset(t[D:D + 1, :], 1.0)

            # one-time zero fill of z-shard pad columns (never written later;
            # keeps AllGather/NaN checks clean)
            zeros64 = pers.tile([128, D], dt.float16, name="zeros64")
            nc.vector.memset(zeros64[:], 0.0)
            for z_shard, n in ((zs_shard, cfg.SRC_SH), (zd_shard, cfg.DST_SH)):
                for k in range(n // CHUNK):
                    nc.sync.dma_start(
                        out=z_shard[k * CHUNK:(k + 1) * CHUNK, D:128],
                        in_=zeros64[:])

            def encoder(xT, w_t, b_t, h_out, n):
                for j0 in range(0, n, WIN):
                    w = min(WIN, n - j0)
                    xs = work.tile([D, WIN], dt.float32, tag="xs")
                    nc.sync.dma_start(out=xs[:, :w], in_=xT[:, j0:j0 + w])
                    ps = ps_mlp.tile([D, WIN], dt.float32, tag="mlp")
                    nc.tensor.matmul(out=ps[:, :w], lhsT=w_t[:], rhs=xs[:, :w],
                                     start=True, stop=True)
                    nc.scalar.activation(out=h_out[0:D, j0:j0 + w],
                                         in_=ps[:, :w], func=RELU, bias=b_t[:])

            REPS = reps

            def z_phase(h_in, wbm, z_shard, n):
                for k in range(n // CHUNK):
                    ps = ps_mlp.tile([CHUNK, D], dt.float32, tag="mlp")
                    nc.tensor.matmul(
                        out=ps[:], lhsT=h_in[0:D + 1, k * CHUNK:(k + 1) * CHUNK],
                        rhs=wbm[:], start=True, stop=True)
                    zs = work.tile([CHUNK, D], dt.float16, tag="zstage")
                    nc.vector.tensor_scalar_max(out=zs[:], in0=ps[:], scalar1=0.0)
                    nc.sync.dma_start(
                        out=z_shard[k * CHUNK:(k + 1) * CHUNK, 0:D], in_=zs[:])

            gather_count = [0]

            def sweep(plan, z_full, idx_dram, dw_t, agg_t, table_rows, shard):
                nc.vector.memset(agg_t[:], 0.0)
                gathers = plan["gathers"]
                gtiles = []
                for gno, (q, s0, nsl) in enumerate(gathers):
                    it = idxg.tile([128, nsl // 16], dt.int16, tag="idxg")
                    nc.sync.dma_start(
                        out=it[:], in_=idx_dram[:, s0 // 16:(s0 + nsl) // 16])
                    gt = gath.tile([128, (nsl // CHUNK) * 128], dt.float16,
                                   tag="gt")
                    nc.gpsimd.dma_gather(
                        gt[:].rearrange("p (b e) -> p b e", e=128),
                        z_full[q * table_rows:(q + 1) * table_rows, :],
                        it[:], nsl, nsl, 128, single_packet=False,
                        queue_num=gather_count[0] % 4)
                    gather_count[0] += 1
                    gtiles.append((s0, nsl, gt))

                SB = 8  # chunks per S-build batch
                gi = 0
                chunk0 = 0  # global chunk cursor
                s_tile = None
                for (q, w, ks) in plan["visits"]:
                    nvis = sum(ks)
                    ps = ps_agg.tile([D, WIN], dt.float32, tag="agg")
                    nc.vector.memset(ps[:], 0.0)
                    done = 0
                    for si, kk in enumerate(ks):
                        for _ in range(kk):
                            c = chunk0
                            # S batch
                            if c % SB == 0:
                                nb = min(SB, dw_t.shape[1] - c)
                                s_tile = spool.tile([128, nb * SUB], dt.float16,
                                                    tag="s")
                                nc.vector.tensor_tensor(
                                    out=s_tile[:].rearrange(
                                        "p (b e) -> p b e", e=SUB),
                                    in0=dw_t[:, c:c + nb, None].to_broadcast(
                                        [128, nb, SUB]),
                                    in1=iota_t[:, None, :].to_broadcast(
                                        [128, nb, SUB]),
                                    op=EQ)
                            # gather tile & block for this chunk
                            s0, nsl, gt = gtiles[gi]
                            if c * CHUNK >= s0 + nsl:
                                gi += 1
                                s0, nsl, gt = gtiles[gi]
                            blk = (c * CHUNK - s0) // CHUNK
                            g3 = gt[:].rearrange("p (b e) -> p b e", e=128)
                            done += 1
                            nc.tensor.matmul(
                                out=ps[:, si * SUB:(si + 1) * SUB],
                                lhsT=g3[:, blk, 0:D],
                                rhs=s_tile[:].rearrange(
                                    "p (b e) -> p b e", e=SUB)[:, c % SB, :],
                                start=False, stop=(done == nvis),
                                skip_group_check=True)
                            chunk0 += 1
                    hi = min((w + 1) * WIN, shard)
                    nc.vector.tensor_add(
                        out=agg_t[:, w * WIN:hi], in0=agg_t[:, w * WIN:hi],
                        in1=ps[:, :hi - w * WIN])

            def update(h_io, agg_t, wu, bu, n):
                for j0 in range(0, n, WIN):
                    w = min(WIN, n - j0)
                    rt = work.tile([2 * D, WIN], dt.float16, tag="updrhs")
                    nc.vector.tensor_copy(out=rt[0:D, :w],
                                          in_=h_io[0:D, j0:j0 + w])
                    nc.vector.tensor_copy(out=rt[D:2 * D, :w],
                                          in_=agg_t[:, j0:j0 + w])
                    ps = ps_mlp.tile([D, WIN], dt.float32, tag="mlp")
                    nc.tensor.matmul(out=ps[:, :w], lhsT=wu[:], rhs=rt[:, :w],
                                     start=True, stop=True)
                    nc.scalar.activation(out=h_io[0:D, j0:j0 + w],
                                         in_=ps[:, :w], func=RELU, bias=bu[:])

            for _rep in range(REPS):
              encoder(xT_src, w_enc_s, b_enc_s, h_s, cfg.SRC_SH)
              encoder(xT_dst, w_enc_d, b_enc_d, h_d, cfg.DST_SH)
              for l in range(L):
                z_phase(h_s, wbm_t["sd", l], zs_shard, cfg.SRC_SH)
                nc.gpsimd.collective_compute(
                    "AllGather", mybir.AluOpType.bypass, replica_groups=rg,
                    ins=[zs_shard.opt()], outs=[zs_full.opt()])
                sweep(plan_sd, zs_full, idx_sd, dw_sd_t, agg_d,
                      cfg.TAB_S, cfg.DST_SH)
                update(h_d, agg_d, wu_t["dst", l], bu_t["dst", l], cfg.DST_SH)

                if l == L - 1:
                    nc.sync.dma_start(out=out_hd[:], in_=h_d[0:D, :])
                    break

                z_phase(h_d, wbm_t["ds", l], zd_shard, cfg.DST_SH)
                nc.gpsimd.collective_compute(
                    "AllGather", mybir.AluOpType.bypass, replica_groups=rg,
                    ins=[zd_shard.opt()], outs=[zd_full.opt()])
                sweep(plan_ds, zd_full, idx_ds, dw_ds_t, agg_s,
                      cfg.TAB_D, cfg.SRC_SH)
                update(h_s, agg_s, wu_t["src", l], bu_t["src", l], cfg.SRC_SH)

    nc.compile()
    return nc


def make_in_maps(cfg, host):
    shared = dict(
        Win_src=host["Win_src"], Win_dst=host["Win_dst"],
        bin_src=host["bin_src"], bin_dst=host["bin_dst"],
        Wbm_sd=host["Wbm_sd"], Wbm_ds=host["Wbm_ds"],
        Wu_dst=host["Wu_dst"], Wu_src=host["Wu_src"],
        bu_dst=host["bu_dst"], bu_src=host["bu_src"],
        iota=host["iota"],
    )
    maps = []
    for c in range(N_CORES):
        m = dict(shared)
        m["xT_src"] = host["xsT"][c]
        m["xT_dst"] = host["xdT"][c]
        m["idx_sd"] = host["plan_sd"]["idx16"][c]
        m["dw_sd"] = host["plan_sd"]["dw"][c]
        m["idx_ds"] = host["plan_ds"]["idx16"][c]
        m["dw_ds"] = host["plan_ds"]["dw"][c]
        maps.append(m)
    return maps


def kernel(**inputs) -> np.ndarray:
    cfg = REAL_CFG
    host = _host_prep(cfg, inputs)
    nc = _build_nc(cfg, host)
    from concourse.bass_utils import run_bass_kernel_spmd
    res = run_bass_kernel_spmd(nc, make_in_maps(cfg, host),
                               core_ids=list(range(N_CORES)))
    nd = np.asarray(inputs["x_dst"]).shape[0]
    out = np.concatenate([res.results[c]["out_hd"].T for c in range(N_CORES)],
                         axis=0)[:nd]
    return out.astype(np.float32)



# revision 5
# speedup vs baseline: 2.3232x; 2.3232x over previous
"""Bipartite GNN message passing on 8 Trainium2 NeuronCores.

Math reformulation: relu(h[idx] @ W + b) == relu(h @ W + b)[idx], so each
direction-layer is: per-node message MLP (z) -> gather z rows by edge ->
segment-sum -> update MLP.  Sharding: aggregation-side nodes are split into
8 contiguous ranges (one per core); each core owns ALL edges targeting its
range, so it computes complete aggregates locally (no AllReduce).  Only the
small per-shard z tensors are AllGathered (fp16) each direction-layer, in 2
pipelined pieces so gathers start before the full AllGather lands.

Segment-sum on the tensor engine: edges sorted by (window512, table,
sub128); for each 128-edge chunk a one-hot S[e,j] = (dst_local[e] == j)
matrix (DVE is_equal against an iota tile) turns the segment sum into
psum[64f, 128dst] += gathered_z[128e, 64f].T @ S[128e, 128dst], accumulated
in a PSUM bank per 512-dst window (start=True clears, no memset) and
flushed once per window into a fused update-MLP (2 accumulating matmuls +
relu activation straight back into h).

Gathers use dma_gather (SWDGE): int16 indices into <=25088-row table slices
of the AllGathered z pieces, 256B padded fp16 rows.  Descriptor generation
runs on a Q7 core pair selected by queue_num, so gathers are issued
round-robin on queues 0-3 with 6 gather buffers to keep 4 desc-gens in
flight (this was the baseline bottleneck: ~7.8ns/row on one pair).

SPMD: one NEFF for all 8 cores; the chunk schedule is the max over cores;
cores pad with (idx=0, dst_local=sentinel) edges that contribute zero.
"""
import numpy as np

D = 64
CHUNK = 128
SUB = 128
WIN = 512
SENT = 999.0
N_CORES = 8
GCAP = 5120  # max slots per dma_gather
SB = 8       # chunks per S-build batch
NQ = 4       # SWDGE queues


class Cfg:
    def __init__(self, ns_pad, nd_pad):
        self.NS_PAD, self.ND_PAD = ns_pad, nd_pad
        self.SRC_SH, self.DST_SH = ns_pad // N_CORES, nd_pad // N_CORES
        # gather-side piece/table geometry
        self.S_HALF = self.SRC_SH // 2           # rows/core per AG piece (src)
        self.D_HALF = self.DST_SH // 2           # rows/core per AG piece (dst)
        self.S_PIECE = self.S_HALF * N_CORES     # rows per src piece tensor
        self.D_PIECE = self.D_HALF * N_CORES     # rows per dst piece tensor
        self.S_TAB = self.S_PIECE // 2           # src table = half piece
        self.D_TAB = self.D_PIECE                # dst table = whole piece
        assert self.S_TAB <= 32767 and self.D_TAB <= 32767
        assert self.SRC_SH % CHUNK == 0 and self.DST_SH % CHUNK == 0


REAL_CFG = Cfg(100352, 50176)


def _src_table_map(cfg, g):
    """src node g -> (table q in 0..3, row in table).  piece p = q//2."""
    c = g // cfg.SRC_SH
    off = g % cfg.SRC_SH
    p = off // cfg.S_HALF
    q = p * 2 + c // 4
    row = (c % 4) * cfg.S_HALF + (off % cfg.S_HALF)
    return q, row


def _dst_table_map(cfg, g):
    """dst node g -> (table q in 0..1, row in table).  piece p = q."""
    c = g // cfg.DST_SH
    off = g % cfg.DST_SH
    p = off // cfg.D_HALF
    row = c * cfg.D_HALF + (off % cfg.D_HALF)
    return p, row


def _build_plan(cfg, gather_idx, seg_idx, table_map, n_tab, shard):
    """SPMD-uniform edge plan for one direction.

    Slot order: (window512, table q, sub128, chunk).  Returns per-core idx16
    [128, TOT/16] int16 and dw [128, TOT/128] fp16, plus uniform
    windows [(w, [(q, si, k)])] and gathers [(q, slot0, nsl)].
    """
    n_w = (shard + WIN - 1) // WIN
    n_si_tot = shard // SUB  # SUB divides shard
    q_all, row_all = table_map(cfg, gather_idx)
    core_of = seg_idx // shard
    per_core = []
    nsi_of_w = [min(WIN // SUB, n_si_tot - w * (WIN // SUB)) for w in range(n_w)]
    Kmax = np.zeros((n_w, n_tab, WIN // SUB), np.int64)
    for c in range(N_CORES):
        m = core_of == c
        row = row_all[m]
        q = q_all[m]
        s = seg_idx[m] - c * shard
        w = s // WIN
        si = (s % WIN) // SUB
        key = (w * n_tab + q) * (WIN // SUB) + si
        order = np.argsort(key, kind="stable")
        row, q, s, w, si = row[order], q[order], s[order], w[order], si[order]
        per_core.append((row, s % SUB, w, q, si))
        cnt = np.zeros((n_w, n_tab, WIN // SUB), np.int64)
        np.add.at(cnt, (w, q, si), 1)
        Kmax = np.maximum(Kmax, (cnt + CHUNK - 1) // CHUNK)

    # every (w, si) needs >=1 chunk so the PSUM column range is written
    for w in range(n_w):
        for si in range(nsi_of_w[w]):
            if Kmax[w, :, si].sum() == 0:
                Kmax[w, 0, si] = 1

    slots_per_group = Kmax * CHUNK
    starts = np.zeros_like(slots_per_group)
    total = 0
    for w in range(n_w):
        for q in range(n_tab):
            for si in range(nsi_of_w[w]):
                starts[w, q, si] = total
                total += int(slots_per_group[w, q, si])
    assert total % CHUNK == 0

    idx16_list, dw_list = [], []
    for c in range(N_CORES):
        row, dwv, w, q, si = per_core[c]
        G = np.zeros(total, np.int32)
        DW = np.full(total, SENT, np.float32)
        kk = (w * n_tab + q) * (WIN // SUB) + si
        bounds = np.flatnonzero(np.diff(kk)) + 1
        for grp in np.split(np.arange(len(row)), bounds):
            if len(grp) == 0:
                continue
            ww, qq, sg = int(w[grp[0]]), int(q[grp[0]]), int(si[grp[0]])
            st = int(starts[ww, qq, sg])
            n = len(grp)
            G[st:st + n] = row[grp]
            DW[st:st + n] = dwv[grp]
        i16 = np.empty((128, total // 16), np.int16)
        base = G.astype(np.int16).reshape(total // 16, 16).T  # [16, T/16]
        for k in range(8):
            i16[16 * k:16 * (k + 1)] = base
        dw = DW.astype(np.float16).reshape(total // CHUNK, CHUNK).T
        idx16_list.append(i16)
        dw_list.append(np.ascontiguousarray(dw))

    windows = []
    for w in range(n_w):
        groups = []
        for q in range(n_tab):
            for si in range(nsi_of_w[w]):
                k = int(Kmax[w, q, si])
                if k:
                    groups.append((q, si, k))
        windows.append((w, groups))

    gathers = []
    for w in range(n_w):
        for q in range(n_tab):
            lo = int(starts[w, q, 0])
            hi = lo + int(slots_per_group[w, q, :nsi_of_w[w]].sum())
            p = lo
            while p < hi:
                n = min(GCAP, hi - p)
                gathers.append((q, p, n))
                p += n
    return dict(idx16=idx16_list, dw=dw_list, windows=windows, gathers=gathers,
                total=total)


def _host_prep(cfg, inputs):
    f32 = np.float32
    x_src = np.asarray(inputs["x_src"], f32)
    x_dst = np.asarray(inputs["x_dst"], f32)
    src_idx = np.asarray(inputs["src_idx"]).astype(np.int64)
    dst_idx = np.asarray(inputs["dst_idx"]).astype(np.int64)
    L = np.asarray(inputs["W_msg_sd"]).shape[0]

    ns, nd = x_src.shape[0], x_dst.shape[0]
    xs = np.zeros((cfg.NS_PAD, D), f32)
    xs[:ns] = x_src
    xd = np.zeros((cfg.ND_PAD, D), f32)
    xd[:nd] = x_dst

    plan_sd = _build_plan(cfg, src_idx, dst_idx, _src_table_map, 4, cfg.DST_SH)
    plan_ds = _build_plan(cfg, dst_idx, src_idx, _dst_table_map, 2, cfg.SRC_SH)

    def stack_wb(wk, bk):
        w = np.asarray(inputs[wk], f32)
        b = np.asarray(inputs[bk], f32)
        out = np.empty((L, D + 1, D), np.float16)
        out[:, :D] = w.astype(np.float16)
        out[:, D] = b.astype(np.float16)
        return out

    host = dict(
        L=L,
        Wbm_sd=stack_wb("W_msg_sd", "b_msg_sd"),
        Wbm_ds=stack_wb("W_msg_ds", "b_msg_ds"),
        Wu_dst=np.asarray(inputs["W_upd_dst"], f32).astype(np.float16),
        Wu_src=np.asarray(inputs["W_upd_src"], f32).astype(np.float16),
        bu_dst=np.asarray(inputs["b_upd_dst"], f32)[:, :, None],
        bu_src=np.asarray(inputs["b_upd_src"], f32)[:, :, None],
        Win_src=np.asarray(inputs["W_in_src"], f32),
        Win_dst=np.asarray(inputs["W_in_dst"], f32),
        bin_src=np.asarray(inputs["b_in_src"], f32)[:, None],
        bin_dst=np.asarray(inputs["b_in_dst"], f32)[:, None],
        iota=np.tile(np.arange(SUB, dtype=np.float16), (128, 1)),
        xsT=[np.ascontiguousarray(xs[c * cfg.SRC_SH:(c + 1) * cfg.SRC_SH].T)
             for c in range(N_CORES)],
        xdT=[np.ascontiguousarray(xd[c * cfg.DST_SH:(c + 1) * cfg.DST_SH].T)
             for c in range(N_CORES)],
        plan_sd=plan_sd, plan_ds=plan_ds,
    )
    return host


def _build_nc(cfg, host):
    import concourse.bass as bass
    import concourse.tile as tile
    from concourse import bacc, mybir

    dt = mybir.dt
    L = host["L"]
    plan_sd, plan_ds = host["plan_sd"], host["plan_ds"]
    TOT_SD, TOT_DS = plan_sd["total"], plan_ds["total"]

    nc = bacc.Bacc("TRN2", target_bir_lowering=False, debug=False,
                   num_devices=N_CORES, num_swdge_queues=NQ)

    def inp(name, shape, dtype):
        return nc.dram_tensor(name, shape, dtype, kind="ExternalInput").ap()

    xT_src = inp("xT_src", [D, cfg.SRC_SH], dt.float32)
    xT_dst = inp("xT_dst", [D, cfg.DST_SH], dt.float32)
    Win_src = inp("Win_src", [D, D], dt.float32)
    Win_dst = inp("Win_dst", [D, D], dt.float32)
    bin_src = inp("bin_src", [D, 1], dt.float32)
    bin_dst = inp("bin_dst", [D, 1], dt.float32)
    Wbm_sd = inp("Wbm_sd", [L, D + 1, D], dt.float16)
    Wbm_ds = inp("Wbm_ds", [L, D + 1, D], dt.float16)
    Wu_dst = inp("Wu_dst", [L, 2 * D, D], dt.float16)
    Wu_src = inp("Wu_src", [L, 2 * D, D], dt.float16)
    bu_dst = inp("bu_dst", [L, D, 1], dt.float32)
    bu_src = inp("bu_src", [L, D, 1], dt.float32)
    iota_in = inp("iota", [128, SUB], dt.float16)
    idx_sd = inp("idx_sd", [128, TOT_SD // 16], dt.int16)
    dw_sd = inp("dw_sd", [128, TOT_SD // CHUNK], dt.float16)
    idx_ds = inp("idx_ds", [128, TOT_DS // 16], dt.int16)
    dw_ds = inp("dw_ds", [128, TOT_DS // CHUNK], dt.float16)
    out_hd = nc.dram_tensor("out_hd", [D, cfg.DST_SH], dt.float16,
                            kind="ExternalOutput").ap()

    # per-piece z shard (local) and AllGathered piece tensors (shared)
    zs_sh = [nc.dram_tensor(f"zs_sh{p}", [cfg.S_HALF, 128], dt.float16).ap()
             for p in range(2)]
    zd_sh = [nc.dram_tensor(f"zd_sh{p}", [cfg.D_HALF, 128], dt.float16).ap()
             for p in range(2)]
    zs_pc = [nc.dram_tensor(f"zs_pc{p}", [cfg.S_PIECE, 128], dt.float16,
                            addr_space="Shared").ap() for p in range(2)]
    zd_pc = [nc.dram_tensor(f"zd_pc{p}", [cfg.D_PIECE, 128], dt.float16,
                            addr_space="Shared").ap() for p in range(2)]

    RELU = mybir.ActivationFunctionType.Relu
    EQ = mybir.AluOpType.is_equal
    rg = [list(range(N_CORES))]

    with tile.TileContext(nc) as tc:
        from contextlib import ExitStack
        with ExitStack() as ctx:
            pers = ctx.enter_context(tc.tile_pool(name="pers", bufs=1))
            ps_agg = ctx.enter_context(
                tc.tile_pool(name="psagg", bufs=4, space="PSUM"))
            ps_mlp = ctx.enter_context(
                tc.tile_pool(name="psmlp", bufs=3, space="PSUM"))
            gath = ctx.enter_context(tc.tile_pool(name="gath", bufs=6))
            idxg = ctx.enter_context(tc.tile_pool(name="idxg", bufs=8))
            spool = ctx.enter_context(tc.tile_pool(name="spool", bufs=4))
            work = ctx.enter_context(tc.tile_pool(name="work", bufs=4))

            h_s = pers.tile([D + 1, cfg.SRC_SH], dt.float16, name="h_s")
            h_d = pers.tile([D + 1, cfg.DST_SH], dt.float16, name="h_d")
            iota_t = pers.tile([128, SUB], dt.float16)
            dw_sd_t = pers.tile([128, TOT_SD // CHUNK], dt.float16)
            dw_ds_t = pers.tile([128, TOT_DS // CHUNK], dt.float16)

            nc.sync.dma_start(out=iota_t[:], in_=iota_in[:])
            nc.sync.dma_start(out=dw_sd_t[:], in_=dw_sd[:])
            nc.sync.dma_start(out=dw_ds_t[:], in_=dw_ds[:])

            w_enc_s = pers.tile([D, D], dt.float32)
            w_enc_d = pers.tile([D, D], dt.float32)
            b_enc_s = pers.tile([D, 1], dt.float32)
            b_enc_d = pers.tile([D, 1], dt.float32)
            nc.sync.dma_start(out=w_enc_s[:], in_=Win_src[:])
            nc.sync.dma_start(out=w_enc_d[:], in_=Win_dst[:])
            nc.sync.dma_start(out=b_enc_s[:], in_=bin_src[:])
            nc.sync.dma_start(out=b_enc_d[:], in_=bin_dst[:])

            wbm_t, wu_t, bu_t = {}, {}, {}
            for l in range(L):
                for key, src in (("sd", Wbm_sd), ("ds", Wbm_ds)):
                    t = pers.tile([D + 1, D], dt.float16, name=f"wbm_{key}{l}")
                    nc.sync.dma_start(out=t[:], in_=src[l])
                    wbm_t[key, l] = t
                for key, src in (("dst", Wu_dst), ("src", Wu_src)):
                    th = pers.tile([D, D], dt.float16, name=f"wuh_{key}{l}")
                    ta = pers.tile([D, D], dt.float16, name=f"wua_{key}{l}")
                    nc.sync.dma_start(out=th[:], in_=src[l, 0:D, :])
                    nc.sync.dma_start(out=ta[:], in_=src[l, D:2 * D, :])
                    wu_t[key, l] = (th, ta)
                for key, src in (("dst", bu_dst), ("src", bu_src)):
                    t = pers.tile([D, 1], dt.float32, name=f"bu_{key}{l}")
                    nc.sync.dma_start(out=t[:], in_=src[l])
                    bu_t[key, l] = t

            for t in (h_s, h_d):
                nc.vector.memset(t[D:D + 1, :], 1.0)

            # one-time zero fill of z-shard pad columns (never written later)
            zeros64 = pers.tile([128, D], dt.float16, name="zeros64")
            nc.vector.memset(zeros64[:], 0.0)
            for z_list, half in ((zs_sh, cfg.S_HALF), (zd_sh, cfg.D_HALF)):
                for z in z_list:
                    for k in range(0, half, CHUNK):
                        cw = min(CHUNK, half - k)
                        nc.sync.dma_start(out=z[k:k + cw, D:128],
                                          in_=zeros64[:cw, :])

            def encoder(xT, w_t, b_t, h_out, n):
                for j0 in range(0, n, WIN):
                    w = min(WIN, n - j0)
                    xs = work.tile([D, WIN], dt.float32, tag="xs")
                    nc.sync.dma_start(out=xs[:, :w], in_=xT[:, j0:j0 + w])
                    ps = ps_mlp.tile([D, WIN], dt.float32, tag="mlp")
                    nc.tensor.matmul(out=ps[:, :w], lhsT=w_t[:], rhs=xs[:, :w],
                                     start=True, stop=True)
                    nc.scalar.activation(out=h_out[0:D, j0:j0 + w],
                                         in_=ps[:, :w], func=RELU, bias=b_t[:])

            def z_piece(h_in, wbm, z_sh, half, p):
                r0 = p * half
                for k in range(0, half, CHUNK):
                    cw = min(CHUNK, half - k)
                    ps = ps_mlp.tile([CHUNK, D], dt.float32, tag="mlp")
                    nc.tensor.matmul(
                        out=ps[:cw, :],
                        lhsT=h_in[0:D + 1, r0 + k:r0 + k + cw],
                        rhs=wbm[:], start=True, stop=True)
                    zs = work.tile([CHUNK, D], dt.float16, tag="zstage")
                    nc.vector.tensor_scalar_max(out=zs[:cw, :], in0=ps[:cw, :],
                                                scalar1=0.0)
                    nc.sync.dma_start(out=z_sh[k:k + cw, 0:D], in_=zs[:cw, :])

            gq = [0]

            def sweep(plan, pieces, tab_of, idx_dram, dw_t, h_io, wu, bu,
                      shard, last=False):
                gathers = plan["gathers"]
                gtiles = []
                for (q, s0, nsl) in gathers:
                    it = idxg.tile([128, nsl // 16], dt.int16, tag="idxg")
                    nc.sync.dma_start(
                        out=it[:], in_=idx_dram[:, s0 // 16:(s0 + nsl) // 16])
                    gt = gath.tile([128, nsl], dt.float16, tag="gt")
                    nc.gpsimd.dma_gather(
                        gt[:].rearrange("p (b e) -> p b e", e=128),
                        tab_of(q), it[:], nsl, nsl, 128, single_packet=False,
                        queue_num=gq[0] % NQ)
                    gq[0] += 1
                    gtiles.append((s0, nsl, gt))

                gi = 0
                c = 0  # global chunk cursor
                s_tile = None
                n_w = len(plan["windows"])
                for (w, groups) in plan["windows"]:
                    nvis = sum(k for (_, _, k) in groups)
                    w0 = w * WIN
                    ww = min(WIN, shard - w0)
                    ps = ps_agg.tile([D, WIN], dt.float32, tag="agg")
                    done = 0
                    for (q, si, kk) in groups:
                        for _ in range(kk):
                            if c % SB == 0:
                                nb = min(SB, dw_t.shape[1] - c)
                                s_tile = spool.tile([128, nb * SUB],
                                                    dt.float16, tag="s")
                                nc.vector.tensor_tensor(
                                    out=s_tile[:].rearrange(
                                        "p (b e) -> p b e", e=SUB),
                                    in0=dw_t[:, c:c + nb, None].to_broadcast(
                                        [128, nb, SUB]),
                                    in1=iota_t[:, None, :].to_broadcast(
                                        [128, nb, SUB]),
                                    op=EQ)
                            s0, nsl, gt = gtiles[gi]
                            if c * CHUNK >= s0 + nsl:
                                gi += 1
                                s0, nsl, gt = gtiles[gi]
                            blk = (c * CHUNK - s0) // CHUNK
                            g3 = gt[:].rearrange("p (b e) -> p b e", e=128)
                            nc.tensor.matmul(
                                out=ps[:, si * SUB:(si + 1) * SUB],
                                lhsT=g3[:, blk, 0:D],
                                rhs=s_tile[:].rearrange(
                                    "p (b e) -> p b e", e=SUB)[:, c % SB, :],
                                start=(done == 0), stop=(done == nvis - 1),
                                skip_group_check=True)
                            done += 1
                            c += 1
                    # fused update for this window
                    ag = work.tile([D, WIN], dt.float16, tag="aggstage")
                    nc.vector.tensor_copy(out=ag[:, :ww], in_=ps[:, :ww])
                    psu = ps_mlp.tile([D, WIN], dt.float32, tag="mlp")
                    nc.tensor.matmul(out=psu[:, :ww], lhsT=wu[0][:],
                                     rhs=h_io[0:D, w0:w0 + ww],
                                     start=True, stop=False,
                                     skip_group_check=True)
                    nc.tensor.matmul(out=psu[:, :ww], lhsT=wu[1][:],
                                     rhs=ag[:, :ww],
                                     start=False, stop=True,
                                     skip_group_check=True)
                    nc.scalar.activation(out=h_io[0:D, w0:w0 + ww],
                                         in_=psu[:, :ww], func=RELU,
                                         bias=bu[:])
                    if last:
                        nc.sync.dma_start(out=out_hd[:, w0:w0 + ww],
                                          in_=h_io[0:D, w0:w0 + ww])

            def sd_tab(q):
                return zs_pc[q // 2][(q % 2) * cfg.S_TAB:
                                    (q % 2 + 1) * cfg.S_TAB, :]

            def ds_tab(q):
                return zd_pc[q]

            encoder(xT_src, w_enc_s, b_enc_s, h_s, cfg.SRC_SH)
            encoder(xT_dst, w_enc_d, b_enc_d, h_d, cfg.DST_SH)
            for l in range(L):
                for p in range(2):
                    z_piece(h_s, wbm_t["sd", l], zs_sh[p], cfg.S_HALF, p)
                    nc.gpsimd.collective_compute(
                        "AllGather", mybir.AluOpType.bypass,
                        replica_groups=rg,
                        ins=[zs_sh[p].opt()], outs=[zs_pc[p].opt()])
                sweep(plan_sd, zs_pc, sd_tab, idx_sd, dw_sd_t, h_d,
                      wu_t["dst", l], bu_t["dst", l], cfg.DST_SH,
                      last=(l == L - 1))
                if l == L - 1:
                    break
                for p in range(2):
                    z_piece(h_d, wbm_t["ds", l], zd_sh[p], cfg.D_HALF, p)
                    nc.gpsimd.collective_compute(
                        "AllGather", mybir.AluOpType.bypass,
                        replica_groups=rg,
                        ins=[zd_sh[p].opt()], outs=[zd_pc[p].opt()])
                sweep(plan_ds, zd_pc, ds_tab, idx_ds, dw_ds_t, h_s,
                      wu_t["src", l], bu_t["src", l], cfg.SRC_SH)

    nc.compile()
    return nc


def make_in_maps(cfg, host):
    shared = dict(
        Win_src=host["Win_src"], Win_dst=host["Win_dst"],
        bin_src=host["bin_src"], bin_dst=host["bin_dst"],
        Wbm_sd=host["Wbm_sd"], Wbm_ds=host["Wbm_ds"],
        Wu_dst=host["Wu_dst"], Wu_src=host["Wu_src"],
        bu_dst=host["bu_dst"], bu_src=host["bu_src"],
        iota=host["iota"],
    )
    maps = []
    for c in range(N_CORES):
        m = dict(shared)
        m["xT_src"] = host["xsT"][c]
        m["xT_dst"] = host["xdT"][c]
        m["idx_sd"] = host["plan_sd"]["idx16"][c]
        m["dw_sd"] = host["plan_sd"]["dw"][c]
        m["idx_ds"] = host["plan_ds"]["idx16"][c]
        m["dw_ds"] = host["plan_ds"]["dw"][c]
        maps.append(m)
    return maps


def kernel(**inputs) -> np.ndarray:
    cfg = REAL_CFG
    host = _host_prep(cfg, inputs)
    nc = _build_nc(cfg, host)
    from concourse.bass_utils import run_bass_kernel_spmd
    res = run_bass_kernel_spmd(nc, make_in_maps(cfg, host),
                               core_ids=list(range(N_CORES)))
    nd = np.asarray(inputs["x_dst"]).shape[0]
    out = np.concatenate([res.results[c]["out_hd"].T for c in range(N_CORES)],
                         axis=0)[:nd]
    return out.astype(np.float32)
